# revision 1
# baseline (speedup 1.0000x reference)
"""Self-contained Trainium kernel for nn_Attention_7662221656252.

Strategy: pure data-parallel over batch (B=16 -> 2 per core across 8
NeuronCores) via jax.pmap.  All convolutions are expressed as shifted-window
einsums and the channel-axis rfft/irfft as small precomputed DFT matmuls so
the graph lowers to plain matmul/elementwise ops.  Falls back to CPU jax,
then pure numpy, if device execution is unavailable.
"""

import numpy as np

B, DIM, H, W, HEADS = 16, 256, 64, 64, 8
C2 = DIM // 2 // HEADS          # 16
CF = C2 // 2 + 1                # 9
DC = DIM // 4                   # 64
NCORES = 8


def _dft_mats():
    c = np.arange(C2)
    f = np.arange(CF)
    ang = 2.0 * np.pi * np.outer(f, c) / C2          # [CF, C2]
    Fr = np.cos(ang).astype(np.float32)
    Fi = (-np.sin(ang)).astype(np.float32)
    w = np.where((f == 0) | (f == C2 // 2), 1.0, 2.0).astype(np.float32)
    angb = 2.0 * np.pi * np.outer(c, f) / C2         # [C2, CF]
    Br = (w[None, :] * np.cos(angb) / C2).astype(np.float32)
    Bi = (-w[None, :] * np.sin(angb) / C2).astype(np.float32)
    return Fr, Fi, Br, Bi


def _forward(xp, x, pc3a_w, hm_conv1_w, hm_proj2_w, hm_proj2_b, pc5_w,
             hm_conv2_w, fuse_w, qkv_pc3_w, qkv_w, proj_w, proj_b,
             temp1, temp2, tw1, tw2, Fr, Fi, Br, Bi, erf):
    """xp: numpy-like namespace (np or jnp). x: [b,256,64,64]."""
    b = x.shape[0]

    def gelu(t):
        return 0.5 * t * (1.0 + erf(t * np.float32(1.0 / np.sqrt(2.0))))

    def conv1x1(t, wmat, bias=None):
        y = xp.einsum('oc,bchw->bohw', wmat, t)
        if bias is not None:
            y = y + bias[None, :, None, None]
        return y

    def pconv(t, wc, k):
        pad = k // 2
        x0 = t[:, :DC]
        x0p = xp.pad(x0, ((0, 0), (0, 0), (pad, pad), (pad, pad)))
        y = None
        for dy in range(k):
            for dx in range(k):
                contrib = xp.einsum('oc,bchw->bohw', wc[:, :, dy, dx],
                                    x0p[:, :, dy:dy + H, dx:dx + W])
                y = contrib if y is None else y + contrib
        return xp.concatenate([y, t[:, DC:]], axis=1)

    def l2norm(t):
        n = xp.sqrt(xp.sum(t * t, axis=-1, keepdims=True))
        return t / xp.maximum(n, np.float32(1e-12))

    def softmax(t):
        m = xp.max(t, axis=-1, keepdims=True)
        e = xp.exp(t - m)
        return e / xp.sum(e, axis=-1, keepdims=True)

    # ---- HighMixer ----
    cx = gelu(conv1x1(pconv(x, pc3a_w, 3), hm_conv1_w))
    px = gelu(conv1x1(x, hm_proj2_w, hm_proj2_b))
    rx = gelu(conv1x1(pconv(x, pc5_w, 5), hm_conv2_w))
    hx = conv1x1(xp.concatenate([cx, px, rx], axis=1), fuse_w) + x

    # ---- qkv ----
    qkv = conv1x1(pconv(hx, qkv_pc3_w, 3), qkv_w)
    q, k, v = qkv[:, :DIM], qkv[:, DIM:2 * DIM], qkv[:, 2 * DIM:]
    to_heads = lambda t: t.reshape(b, HEADS, DIM // HEADS, H * W)
    q, k, v = to_heads(q), to_heads(k), to_heads(v)
    q, k, v = q[:, :, C2:], k[:, :, C2:], v[:, :, C2:]   # [b,8,16,4096]

    # ---- branch 1: channel attention ----
    q1, k1 = l2norm(q), l2norm(k)
    attn1 = xp.einsum('bhcn,bhdn->bhcd', q1, k1) * temp1
    attn1 = softmax(attn1) * tw2
    out1 = xp.einsum('bhcd,bhdn->bhcn', attn1, v).reshape(b, DIM // 2, H, W)

    # ---- branch 2: FFT-domain channel attention (real matmul form) ----
    qfr = xp.einsum('fc,bhcn->bhfn', Fr, q)
    qfi = xp.einsum('fc,bhcn->bhfn', Fi, q)
    kfr = xp.einsum('fc,bhcn->bhfn', Fr, k)
    kfi = xp.einsum('fc,bhcn->bhfn', Fi, k)
    vfr = xp.einsum('fc,bhcn->bhfn', Fr, v)
    vfi = xp.einsum('fc,bhcn->bhfn', Fi, v)

    qn = xp.sqrt(xp.sum(qfr * qfr + qfi * qfi, axis=-1, keepdims=True))
    qn = xp.maximum(qn, np.float32(1e-12))
    kn = xp.sqrt(xp.sum(kfr * kfr + kfi * kfi, axis=-1, keepdims=True))
    kn = xp.maximum(kn, np.float32(1e-12))
    qfr, qfi = qfr / qn, qfi / qn
    kfr, kfi = kfr / kn, kfi / kn

    ar = (xp.einsum('bhcn,bhdn->bhcd', qfr, kfr)
          - xp.einsum('bhcn,bhdn->bhcd', qfi, kfi)) * temp2
    ai = (xp.einsum('bhcn,bhdn->bhcd', qfr, kfi)
          + xp.einsum('bhcn,bhdn->bhcd', qfi, kfr)) * temp2
    ar = softmax(ar) * tw1
    ai = softmax(ai) * tw1

    lxr = (xp.einsum('bhcd,bhdn->bhcn', ar, vfr)
           - xp.einsum('bhcd,bhdn->bhcn', ai, vfi))
    lxi = (xp.einsum('bhcd,bhdn->bhcn', ar, vfi)
           + xp.einsum('bhcd,bhdn->bhcn', ai, vfr))

    lx = (xp.einsum('cf,bhfn->bhcn', Br, lxr)
          + xp.einsum('cf,bhfn->bhcn', Bi, lxi)).reshape(b, DIM // 2, H, W)

    # ---- fuse + project ----
    out = conv1x1(xp.concatenate([lx, out1], axis=1), proj_w, proj_b)
    return out


def _run_jax_pmap(inputs, Fr, Fi, Br, Bi):
    import jax
    import jax.numpy as jnp
    from jax.scipy.special import erf

    devs = jax.devices()
    n = min(NCORES, len(devs))
    if n < NCORES or B % n != 0:
        raise RuntimeError("not enough devices")

    wnames = ['pc3a_w', 'hm_conv1_w', 'hm_proj2_w', 'hm_proj2_b', 'pc5_w',
              'hm_conv2_w', 'fuse_w', 'qkv_pc3_w', 'qkv_w', 'proj_w',
              'proj_b', 'temp1', 'temp2', 'tw1', 'tw2']

    def f(x_shard, *ws):
        return _forward(jnp, x_shard, *ws, Fr, Fi, Br, Bi, erf)

    pm = jax.pmap(f, in_axes=(0,) + (None,) * (len(wnames) + 4),
                  devices=devs[:n])
    xs = np.ascontiguousarray(
        inputs['x'].reshape(n, B // n, DIM, H, W))
    args = [np.asarray(inputs[k], np.float32) for k in wnames]
    out = pm(xs, *args, Fr, Fi, Br, Bi)
    out = np.asarray(out, np.float32).reshape(B, DIM, H, W)
    return out


def _run_jax_cpu(inputs, Fr, Fi, Br, Bi):
    import jax
    import jax.numpy as jnp
    from jax.scipy.special import erf
    cpu = jax.devices('cpu')[0]
    wnames = ['pc3a_w', 'hm_conv1_w', 'hm_proj2_w', 'hm_proj2_b', 'pc5_w',
              'hm_conv2_w', 'fuse_w', 'qkv_pc3_w', 'qkv_w', 'proj_w',
              'proj_b', 'temp1', 'temp2', 'tw1', 'tw2']

    def f(x, *ws):
        return _forward(jnp, x, *ws, Fr, Fi, Br, Bi, erf)

    jf = jax.jit(f, device=cpu)
    args = [np.asarray(inputs[k], np.float32) for k in wnames]
    out = jf(np.asarray(inputs['x'], np.float32), *args)
    return np.asarray(out, np.float32)


def _run_numpy(inputs, Fr, Fi, Br, Bi):
    try:
        from scipy.special import erf
    except Exception:
        def erf(t):
            # Abramowitz-Stegun 7.1.26, max abs err ~1.5e-7
            sign = np.sign(t)
            a = np.abs(t)
            tt = 1.0 / (1.0 + 0.3275911 * a)
            y = 1.0 - (((((1.061405429 * tt - 1.453152027) * tt)
                         + 1.421413741) * tt - 0.284496736) * tt
                       + 0.254829592) * tt * np.exp(-a * a)
            return sign * y
    wnames = ['pc3a_w', 'hm_conv1_w', 'hm_proj2_w', 'hm_proj2_b', 'pc5_w',
              'hm_conv2_w', 'fuse_w', 'qkv_pc3_w', 'qkv_w', 'proj_w',
              'proj_b', 'temp1', 'temp2', 'tw1', 'tw2']
    args = [np.asarray(inputs[k], np.float32) for k in wnames]
    return _forward(np, np.asarray(inputs['x'], np.float32), *args,
                    Fr, Fi, Br, Bi, erf).astype(np.float32)


def kernel(**inputs):
    Fr, Fi, Br, Bi = _dft_mats()
    try:
        return _run_jax_pmap(inputs, Fr, Fi, Br, Bi)
    except Exception:
        pass
    try:
        return _run_jax_cpu(inputs, Fr, Fi, Br, Bi)
    except Exception:
        pass
    return _run_numpy(inputs, Fr, Fi, Br, Bi)

